# revision 6
# baseline (speedup 1.0000x reference)
"""Trainium2 Bass kernel for nn_ConsistencyMaskFromBoxes.

Computes: loss = WEIGHT * mean(BCEWithLogits(seg_pred * eff, boxes_mask * eff))

Algorithm
---------
For effective images (not is_seg, has boxes), per-pixel BCE with a {0,1}
target t factorizes:
    bce = softplus(l) - l*t
so  sum(bce) = sum(softplus(l)) - sum_{mask} l.

sum(softplus(l)) uses the identity softplus(l) = -ln(sigmoid(-l)):
  * scalar engine: one Sigmoid pass per image (the ONLY full-image ACT
    pass), s = sigmoid(-l) in bf16, s in (0,1).
  * vector engine: one pairwise-product level compresses 2 sigmoids into
    one product p = s_i * s_j in bf16, p in (0,1), no over/underflow.
  * sum(ln p) via the fast-log bit trick: for bf16, ln p ~ (J - K)*ln2/128
    with J the uint16 bit pattern.  The device computes plain integer sums
    of J (gpsimd full reduce to a scalar); the affine map and the mean
    sawtooth correction constant happen on the host.  Measured end-to-end
    error of this scheme on randn data: ~1e-5 relative.

sum_{mask} l: host decomposes each image's box union into DISJOINT rects
(sweep line), so mask = sum_r rowhit[r,y]*colhit[r,x] exactly, and the
row contraction is a PE matmul accumulated over 5 row-tiles into PSUM
[k_pad, 640]; the colhit dot is a vector multiply + reduce, then a gpsimd
full reduce to a scalar.

All per-core results are single f32 scalars on partition 0 -> one 24-byte
single-queue DMA out (fast completion, no multi-queue semaphore dribble).

DMA uses both hardware dispatchers: the SP ring streams image 0 + the
geometry tensors while the Activation ring streams image 1 concurrently.

Sharding: data-parallel over batch, 2 images per core on 8 cores (SPMD,
per-core differences carried entirely by inputs).
"""

import math
import numpy as np
import ml_dtypes

import concourse.bass as bass
import concourse.bacc as bacc
import concourse.mybir as mybir
import concourse.tile as tile
from concourse.bass_utils import run_bass_kernel_spmd

WEIGHT = 0.1
B, M, H, W = 16, 256, 640, 640

# Keep only the sigmoid table set so a single ACT_TABLE_LOAD covers the
# kernel (indices must be preserved — act_func_set_id is the index into
# act_info.json).
_ACT_TABLE_KEEP = "sigmoid_and_others"
_orig_get_activation_tables = None


def _patch_act_tables():
    global _orig_get_activation_tables
    if _orig_get_activation_tables is not None:
        return
    import concourse.hw_specs as hw_specs
    _orig_get_activation_tables = hw_specs.get_activation_tables

    def patched(arch):
        tabs = _orig_get_activation_tables(arch)
        if _ACT_TABLE_KEEP in tabs:
            tabs = {name: (fns if name == _ACT_TABLE_KEEP else set())
                    for name, fns in tabs.items()}
        return tabs

    hw_specs.get_activation_tables = patched
    bacc.get_activation_tables = patched


N_CORES = 8
IPC = B // N_CORES          # images per core
PT = 128                    # SBUF partitions
NT = H // PT                # row tiles per image (5)
NW = NT * W                 # columns per image in SBUF layout (3200)
SEG_DT = mybir.dt.bfloat16
SEG_NP = ml_dtypes.bfloat16

# fast-log constants (bf16): value bits J = 128*E + m, E exponent, m mantissa
# ln p = ln2/128 * (J - 128*127) + ln2*(log2(1+m/128) - m/128)
# mean of the sawtooth term over uniform mantissa: C0 = 1.5 - 1/ln2
_C0 = 1.5 - 1.0 / math.log(2.0)
_KC = 16256.0 - 128.0 * _C0            # J offset incl. mean correction
_LN2_128 = math.log(2.0) / 128.0

_PROG_CACHE: dict[tuple, object] = {}

# test-harness hooks (ignored in normal use): set TRACE=True to profile the
# SPMD launch; the BassKernelResults lands in LAST_RESULT.
TRACE = False
LAST_RESULT = None


# ----------------------------------------------------------------- host prep

def _box_coords(bboxes: np.ndarray, h: int, w: int):
    """Integer box corners, bit-exact float32 math as the reference."""
    bb = bboxes.astype(np.float32)
    cx = bb[:, 0] * np.float32(w)
    cy = bb[:, 1] * np.float32(h)
    bw = bb[:, 2] * np.float32(w)
    bh = bb[:, 3] * np.float32(h)
    two = np.float32(2.0)
    x1 = np.clip(cx - bw / two, 0.0, w - 1).astype(np.int32)
    y1 = np.clip(cy - bh / two, 0.0, h - 1).astype(np.int32)
    x2 = np.clip(cx + bw / two, 0.0, w - 1).astype(np.int32)
    y2 = np.clip(cy + bh / two, 0.0, h - 1).astype(np.int32)
    return x1, y1, x2, y2


def _disjoint_rects(boxes):
    """boxes: list of (x1,y1,x2,y2) inclusive ints. Returns disjoint rects
    (x1,x2,y1,y2) inclusive whose union equals the union of the boxes."""
    if not boxes:
        return []
    edges = sorted(set([b[0] for b in boxes] + [b[2] + 1 for b in boxes]))
    slabs = []
    for i in range(len(edges) - 1):
        xs, xe = edges[i], edges[i + 1]
        active = sorted((b[1], b[3]) for b in boxes if b[0] <= xs and b[2] + 1 >= xe)
        ints = []
        for a, bb in active:
            if ints and a <= ints[-1][1] + 1:
                ints[-1][1] = max(ints[-1][1], bb)
            else:
                ints.append([a, bb])
        if ints:
            slabs.append((xs, xe, tuple(tuple(t) for t in ints)))
    merged = []
    for xs, xe, ints in slabs:
        if merged and merged[-1][1] == xs and merged[-1][2] == ints:
            merged[-1][1] = xe
        else:
            merged.append([xs, xe, ints])
    out = []
    for xs, xe, ints in merged:
        for a, bb in ints:
            out.append((xs, xe - 1, a, bb))
    return out


# ------------------------------------------------------------- device program

def _build_program(k_pad: int, n_chunks: int):
    """SPMD program for one core: IPC images, each with n_chunks groups of
    up to k_pad disjoint rects. Returns compiled Bacc."""
    V = IPC * n_chunks  # virtual (image, chunk) pairs
    C = 2 * IPC + V     # output scalars: 2 J-sums per image, then mask dots
    _patch_act_tables()
    nc = bacc.Bacc("TRN2", target_bir_lowering=False, debug=False)

    # seg is host-transposed to [image, sbuf_partition, row_tile * col] so
    # each partition's DMA payload is contiguous
    seg = nc.dram_tensor("seg", [IPC, PT, NW], SEG_DT, kind="ExternalInput")
    rowhit = nc.dram_tensor("rowhit", [PT, V * NT * k_pad], SEG_DT,
                            kind="ExternalInput")
    colhit = nc.dram_tensor("colhit", [V * k_pad, W], SEG_DT,
                            kind="ExternalInput")
    outv = nc.dram_tensor("outv", [1, C], mybir.dt.float32,
                          kind="ExternalOutput")

    AF = mybir.ActivationFunctionType
    OP = mybir.AluOpType
    U16 = mybir.dt.uint16
    F32 = mybir.dt.float32
    XA = mybir.AxisListType

    # seg DMA chunk edges per image (columns); image 0 finer for early start
    CHUNKS = [[0, 800, 1600, 3200], [0, 1600, 3200]]

    with tile.TileContext(nc) as tc:
        with (
            tc.tile_pool(name="seg", bufs=2) as seg_pool,
            tc.tile_pool(name="sig", bufs=2) as sig_pool,
            tc.tile_pool(name="prod", bufs=2) as prod_pool,
            tc.tile_pool(name="small", bufs=1) as small_pool,
            tc.tile_pool(name="macc", bufs=2) as macc_pool,
            tc.tile_pool(name="scr", bufs=2) as scr_pool,
            tc.tile_pool(name="ps", bufs=2, space="PSUM") as psum_pool,
        ):
            # DMA: SP ring streams image 0 + geometry; Activation ring
            # streams image 1 concurrently (dispatched before the act-table
            # load pass inserts the table load ahead of the first sigmoid).
            seg_ts = []
            for i in range(IPC):
                seg_t = seg_pool.tile([PT, NW], SEG_DT, tag=f"seg{i}")
                eng = nc.sync if i == 0 else nc.scalar
                edges = CHUNKS[min(i, len(CHUNKS) - 1)]
                for lo, hi in zip(edges[:-1], edges[1:]):
                    eng.dma_start(seg_t[:, lo:hi], seg[i][:, lo:hi])
                seg_ts.append(seg_t)
            rh = small_pool.tile([PT, V * NT * k_pad], SEG_DT, tag="rh")
            nc.sync.dma_start(rh[:], rowhit[:])
            ch = small_pool.tile([V * k_pad, W], SEG_DT, tag="ch")
            nc.sync.dma_start(ch[:], colhit[:])

            import concourse.bass_isa as bass_isa
            combo = small_pool.tile([PT, C], F32, tag="combo")
            nc.vector.memset(combo[:], 0.0)

            for i in range(IPC):
                seg_t = seg_ts[i]

                # ---- mask-dot path: PE matmuls per chunk, vector mult +
                #      reduce, gpsimd full reduce to a scalar.
                for c in range(n_chunks):
                    v = i * n_chunks + c
                    ps = psum_pool.tile([k_pad, W], F32, tag="ps")
                    for t in range(NT):
                        lhsT = rh[:, (v * NT + t) * k_pad:(v * NT + t + 1) * k_pad]
                        rhs = seg_t[:, t * W:(t + 1) * W]
                        nc.tensor.matmul(ps[:, 0:512], lhsT, rhs[:, 0:512],
                                         start=(t == 0), stop=(t == NT - 1))
                        nc.tensor.matmul(ps[:, 512:W], lhsT, rhs[:, 512:W],
                                         start=(t == 0), stop=(t == NT - 1))
                    mm_scr = scr_pool.tile([k_pad, W], SEG_DT, tag="mm_scr")
                    nc.vector.tensor_tensor(
                        mm_scr[:], ps[:], ch[v * k_pad:(v + 1) * k_pad, :],
                        op=OP.mult)
                    nc.vector.tensor_reduce(
                        combo[0:k_pad, 2 * IPC + v:2 * IPC + v + 1],
                        mm_scr[:], axis=XA.X, op=OP.add)

                # ---- softplus path: sigmoid chunks tracking the DMA, one
                #      pairwise product level, gpsimd bit-pattern sums.
                sig_t = sig_pool.tile([PT, NW], SEG_DT, tag=f"sig{i}")
                edges = CHUNKS[min(i, len(CHUNKS) - 1)]
                for lo, hi in zip(edges[:-1], edges[1:]):
                    nc.scalar.activation(sig_t[:, lo:hi], seg_t[:, lo:hi],
                                         AF.Sigmoid, scale=-1.0)

                for half in range(2):
                    p1 = prod_pool.tile([PT, 800], SEG_DT, tag=f"p1{half}")
                    lo = 1600 * half
                    nc.vector.tensor_tensor(p1[:], sig_t[:, lo:lo + 800],
                                            sig_t[:, lo + 800:lo + 1600],
                                            op=OP.mult)
                    with nc.allow_low_precision(reason="u16 bit sum in f32"):
                        nc.vector.tensor_reduce(
                            combo[:, 2 * i + half:2 * i + half + 1],
                            p1[:].bitcast(U16), axis=XA.X, op=OP.add)

            allred = small_pool.tile([PT, C], F32, tag="allred")
            nc.gpsimd.partition_all_reduce(allred[:], combo[:], channels=PT,
                                           reduce_op=bass_isa.ReduceOp.add)
            nc.sync.dma_start(outv[:], allred[0:1, :])

    nc.compile()
    return nc


def _get_program(k_pad: int, n_chunks: int):
    key = (k_pad, n_chunks)
    if key not in _PROG_CACHE:
        _PROG_CACHE[key] = _build_program(k_pad, n_chunks)
    return _PROG_CACHE[key]


# -------------------------------------------------------------------- kernel

def kernel(seg_pred: np.ndarray, bboxes: np.ndarray, batch_idx: np.ndarray,
           is_seg: np.ndarray) -> np.ndarray:
    seg_pred = np.asarray(seg_pred, dtype=np.float32)
    bboxes = np.asarray(bboxes, dtype=np.float32)
    batch_idx = np.asarray(batch_idx)
    is_seg = np.asarray(is_seg).astype(bool)
    assert seg_pred.shape == (B, 1, H, W), seg_pred.shape

    x1, y1, x2, y2 = _box_coords(bboxes, H, W)
    per_img = [[] for _ in range(B)]
    for m in range(bboxes.shape[0]):
        bi = int(batch_idx[m])
        if 0 <= bi < B:
            per_img[bi].append((int(x1[m]), int(y1[m]), int(x2[m]), int(y2[m])))

    has_box = np.array([len(p) > 0 for p in per_img])
    eff = (~is_seg) & has_box
    if not (eff.any() and not is_seg.all()):
        return np.float32(0.0)

    rects = [_disjoint_rects(p) for p in per_img]
    k_max = max(len(r) for r in rects)
    n_chunks = max(1, math.ceil(k_max / PT))
    # multiples of 32: engine partition-offset reads of the colhit tile
    # must start at a 32-partition boundary
    k_pad = min(PT, max(32, math.ceil(k_max / n_chunks / 32) * 32))
    V = IPC * n_chunks

    # per-core input arrays
    in_maps = []
    for core in range(N_CORES):
        imgs = [core * IPC + i for i in range(IPC)]
        # [i, p, t*W+w] layout: each SBUF partition's payload is contiguous
        seg_arr = np.ascontiguousarray(
            seg_pred[imgs, 0].reshape(IPC, NT, PT, W).transpose(0, 2, 1, 3)
            .reshape(IPC, PT, NW).astype(SEG_NP))
        rh_arr = np.zeros((PT, V * NT * k_pad), SEG_NP)
        ch_arr = np.zeros((V * k_pad, W), SEG_NP)
        for i, b in enumerate(imgs):
            for r, (rx1, rx2, ry1, ry2) in enumerate(rects[b]):
                c, rr = divmod(r, k_pad)
                v = i * n_chunks + c
                ch_arr[v * k_pad + rr, rx1:rx2 + 1] = 1
                for t in range(NT):
                    lo, hi = max(ry1, t * PT), min(ry2, t * PT + PT - 1)
                    if lo <= hi:
                        col = (v * NT + t) * k_pad + rr
                        rh_arr[lo - t * PT:hi - t * PT + 1, col] = 1
        in_maps.append({"seg": seg_arr, "rowhit": rh_arr, "colhit": ch_arr})

    nc = _get_program(k_pad, n_chunks)
    global LAST_RESULT
    res = run_bass_kernel_spmd(nc, in_maps, list(range(N_CORES)), trace=TRACE)
    LAST_RESULT = res

    # host reduction in float64
    total = 0.0
    log2_full = math.log(2.0) * H * W
    n_comp = PT * (NW // 2)     # compressed elements per image
    for core in range(N_CORES):
        ov = res.results[core]["outv"][0].astype(np.float64)  # [2*IPC + V]
        for i in range(IPC):
            b = core * IPC + i
            if eff[b]:
                jsum = ov[2 * i] + ov[2 * i + 1]
                sp_sum = -(jsum - n_comp * _KC) * _LN2_128
                m_sum = ov[2 * IPC + i * n_chunks:
                           2 * IPC + (i + 1) * n_chunks].sum()
                total += sp_sum - m_sum
            else:
                total += log2_full
    loss = WEIGHT * total / (B * H * W)
    return np.float32(loss)


# revision 7
# speedup vs baseline: 1.0133x; 1.0133x over previous
"""Trainium2 Bass kernel for nn_ConsistencyMaskFromBoxes.

Computes: loss = WEIGHT * mean(BCEWithLogits(seg_pred * eff, boxes_mask * eff))

Algorithm
---------
For effective images (not is_seg, has boxes), per-pixel BCE with a {0,1}
target t factorizes:
    bce = softplus(l) - l*t
so  sum(bce) = sum(softplus(l)) - sum_{mask} l.

sum(softplus(l)) uses the identity softplus(l) = -ln(sigmoid(-l)):
  * scalar engine: one Sigmoid pass per image (the ONLY full-image ACT
    pass), s = sigmoid(-l) in bf16, s in (0,1).
  * vector engine: one pairwise-product level compresses 2 sigmoids into
    one product p = s_i * s_j in bf16, p in (0,1), no over/underflow.
  * sum(ln p) via the fast-log bit trick: for bf16, ln p ~ (J - K)*ln2/128
    with J the uint16 bit pattern.  The device computes plain integer sums
    of J (gpsimd full reduce to a scalar); the affine map and the mean
    sawtooth correction constant happen on the host.  Measured end-to-end
    error of this scheme on randn data: ~1e-5 relative.

sum_{mask} l: host decomposes each image's box union into DISJOINT rects
(sweep line), so mask = sum_r rowhit[r,y]*colhit[r,x] exactly, and the
row contraction is a PE matmul accumulated over 5 row-tiles into PSUM
[k_pad, 640]; the colhit dot is a vector multiply + reduce, then a gpsimd
full reduce to a scalar.

All per-core results are single f32 scalars on partition 0 -> one 24-byte
single-queue DMA out (fast completion, no multi-queue semaphore dribble).

DMA uses both hardware dispatchers: the SP ring streams image 0 + the
geometry tensors while the Activation ring streams image 1 concurrently.

Sharding: data-parallel over batch, 2 images per core on 8 cores (SPMD,
per-core differences carried entirely by inputs).
"""

import math
import numpy as np
import ml_dtypes

import concourse.bass as bass
import concourse.bacc as bacc
import concourse.mybir as mybir
import concourse.tile as tile
from concourse.bass_utils import run_bass_kernel_spmd

WEIGHT = 0.1
B, M, H, W = 16, 256, 640, 640

# Keep only the sigmoid table set so a single ACT_TABLE_LOAD covers the
# kernel (indices must be preserved — act_func_set_id is the index into
# act_info.json).
_ACT_TABLE_KEEP = "sigmoid_and_others"
_orig_get_activation_tables = None


def _patch_act_tables():
    global _orig_get_activation_tables
    if _orig_get_activation_tables is not None:
        return
    import concourse.hw_specs as hw_specs
    _orig_get_activation_tables = hw_specs.get_activation_tables

    def patched(arch):
        tabs = _orig_get_activation_tables(arch)
        if _ACT_TABLE_KEEP in tabs:
            tabs = {name: (fns if name == _ACT_TABLE_KEEP else set())
                    for name, fns in tabs.items()}
        return tabs

    hw_specs.get_activation_tables = patched
    bacc.get_activation_tables = patched


N_CORES = 8
IPC = B // N_CORES          # images per core
PT = 128                    # SBUF partitions
NT = H // PT                # row tiles per image (5)
NW = NT * W                 # columns per image in SBUF layout (3200)
SEG_DT = mybir.dt.bfloat16
SEG_NP = ml_dtypes.bfloat16

# fast-log constants (bf16): value bits J = 128*E + m, E exponent, m mantissa
# ln p = ln2/128 * (J - 128*127) + ln2*(log2(1+m/128) - m/128)
# mean of the sawtooth term over uniform mantissa: C0 = 1.5 - 1/ln2
_C0 = 1.5 - 1.0 / math.log(2.0)
_KC = 16256.0 - 128.0 * _C0            # J offset incl. mean correction
_LN2_128 = math.log(2.0) / 128.0

_PROG_CACHE: dict[tuple, object] = {}

# test-harness hooks (ignored in normal use): set TRACE=True to profile the
# SPMD launch; the BassKernelResults lands in LAST_RESULT.
TRACE = False
LAST_RESULT = None


# ----------------------------------------------------------------- host prep

def _box_coords(bboxes: np.ndarray, h: int, w: int):
    """Integer box corners, bit-exact float32 math as the reference."""
    bb = bboxes.astype(np.float32)
    cx = bb[:, 0] * np.float32(w)
    cy = bb[:, 1] * np.float32(h)
    bw = bb[:, 2] * np.float32(w)
    bh = bb[:, 3] * np.float32(h)
    two = np.float32(2.0)
    x1 = np.clip(cx - bw / two, 0.0, w - 1).astype(np.int32)
    y1 = np.clip(cy - bh / two, 0.0, h - 1).astype(np.int32)
    x2 = np.clip(cx + bw / two, 0.0, w - 1).astype(np.int32)
    y2 = np.clip(cy + bh / two, 0.0, h - 1).astype(np.int32)
    return x1, y1, x2, y2


def _disjoint_rects(boxes):
    """boxes: list of (x1,y1,x2,y2) inclusive ints. Returns disjoint rects
    (x1,x2,y1,y2) inclusive whose union equals the union of the boxes."""
    if not boxes:
        return []
    edges = sorted(set([b[0] for b in boxes] + [b[2] + 1 for b in boxes]))
    slabs = []
    for i in range(len(edges) - 1):
        xs, xe = edges[i], edges[i + 1]
        active = sorted((b[1], b[3]) for b in boxes if b[0] <= xs and b[2] + 1 >= xe)
        ints = []
        for a, bb in active:
            if ints and a <= ints[-1][1] + 1:
                ints[-1][1] = max(ints[-1][1], bb)
            else:
                ints.append([a, bb])
        if ints:
            slabs.append((xs, xe, tuple(tuple(t) for t in ints)))
    merged = []
    for xs, xe, ints in slabs:
        if merged and merged[-1][1] == xs and merged[-1][2] == ints:
            merged[-1][1] = xe
        else:
            merged.append([xs, xe, ints])
    out = []
    for xs, xe, ints in merged:
        for a, bb in ints:
            out.append((xs, xe - 1, a, bb))
    return out


# ------------------------------------------------------------- device program

def _build_program(k_pad: int, n_chunks: int):
    """SPMD program for one core: IPC images, each with n_chunks groups of
    up to k_pad disjoint rects. Returns compiled Bacc."""
    V = IPC * n_chunks  # virtual (image, chunk) pairs
    C = IPC + V         # output scalars: J-sum per image, then mask dots
    _patch_act_tables()
    nc = bacc.Bacc("TRN2", target_bir_lowering=False, debug=False)

    # seg is host-transposed to [image, sbuf_partition, row_tile * col] so
    # each partition's DMA payload is contiguous
    seg = nc.dram_tensor("seg", [IPC, PT, NW], SEG_DT, kind="ExternalInput")
    rowhit = nc.dram_tensor("rowhit", [PT, V * NT * k_pad], SEG_DT,
                            kind="ExternalInput")
    colhit = nc.dram_tensor("colhit", [V * k_pad, W], SEG_DT,
                            kind="ExternalInput")
    outv = nc.dram_tensor("outv", [1, C], mybir.dt.float32,
                          kind="ExternalOutput")

    AF = mybir.ActivationFunctionType
    OP = mybir.AluOpType
    U16 = mybir.dt.uint16
    F32 = mybir.dt.float32
    XA = mybir.AxisListType

    # seg DMA chunk edges per image (columns); image 0 finer for early start
    CHUNKS = [[0, 800, 1600, 3200], [0, 1600, 3200]]

    with tile.TileContext(nc) as tc:
        with (
            tc.tile_pool(name="seg", bufs=2) as seg_pool,
            tc.tile_pool(name="sig", bufs=2) as sig_pool,
            tc.tile_pool(name="prod", bufs=2) as prod_pool,
            tc.tile_pool(name="small", bufs=1) as small_pool,
            tc.tile_pool(name="macc", bufs=2) as macc_pool,
            tc.tile_pool(name="scr", bufs=2) as scr_pool,
            tc.tile_pool(name="ps", bufs=2, space="PSUM") as psum_pool,
        ):
            # seg DMAs on the SP ring: image 0 chunks, then the small
            # geometry tensors, then image 1 — queue FIFO order matches the
            # compute order so nothing jumps the line.
            seg_ts = []
            for i in range(IPC):
                seg_t = seg_pool.tile([PT, NW], SEG_DT, tag=f"seg{i}")
                edges = CHUNKS[min(i, len(CHUNKS) - 1)]
                for lo, hi in zip(edges[:-1], edges[1:]):
                    nc.sync.dma_start(seg_t[:, lo:hi], seg[i][:, lo:hi])
                seg_ts.append(seg_t)
                if i == 0:
                    rh = small_pool.tile([PT, V * NT * k_pad], SEG_DT, tag="rh")
                    nc.sync.dma_start(rh[:], rowhit[:])
                    ch = small_pool.tile([V * k_pad, W], SEG_DT, tag="ch")
                    nc.sync.dma_start(ch[:], colhit[:])

            combo = small_pool.tile([1, C], F32, tag="combo")

            for i in range(IPC):
                seg_t = seg_ts[i]

                # ---- mask-dot path: PE matmuls per chunk, vector mult +
                #      reduce, gpsimd full reduce to a scalar.
                for c in range(n_chunks):
                    v = i * n_chunks + c
                    ps = psum_pool.tile([k_pad, W], F32, tag="ps")
                    for t in range(NT):
                        lhsT = rh[:, (v * NT + t) * k_pad:(v * NT + t + 1) * k_pad]
                        rhs = seg_t[:, t * W:(t + 1) * W]
                        nc.tensor.matmul(ps[:, 0:512], lhsT, rhs[:, 0:512],
                                         start=(t == 0), stop=(t == NT - 1))
                        nc.tensor.matmul(ps[:, 512:W], lhsT, rhs[:, 512:W],
                                         start=(t == 0), stop=(t == NT - 1))
                    mm_scr = scr_pool.tile([k_pad, W], SEG_DT, tag="mm_scr")
                    nc.vector.tensor_tensor(
                        mm_scr[:], ps[:], ch[v * k_pad:(v + 1) * k_pad, :],
                        op=OP.mult)
                    nc.gpsimd.tensor_reduce(
                        combo[:, IPC + v:IPC + v + 1], mm_scr[:],
                        axis=XA.XYZWC, op=OP.add)

                # ---- softplus path: sigmoid chunks tracking the DMA, one
                #      pairwise product level, gpsimd bit-pattern sums.
                sig_t = sig_pool.tile([PT, NW], SEG_DT, tag=f"sig{i}")
                edges = CHUNKS[min(i, len(CHUNKS) - 1)]
                for lo, hi in zip(edges[:-1], edges[1:]):
                    nc.scalar.activation(sig_t[:, lo:hi], seg_t[:, lo:hi],
                                         AF.Sigmoid, scale=-1.0)

                p2s = []
                for half in range(2):
                    p1 = prod_pool.tile([PT, 800], SEG_DT, tag=f"p1{half}")
                    lo = 1600 * half
                    nc.vector.tensor_tensor(p1[:], sig_t[:, lo:lo + 800],
                                            sig_t[:, lo + 800:lo + 1600],
                                            op=OP.mult)
                    p2 = prod_pool.tile([PT, 400], SEG_DT, tag=f"p2{half}")
                    nc.gpsimd.tensor_tensor(p2[:], p1[:, 0:400],
                                            p1[:, 400:800], op=OP.mult)
                    p2s.append(p2)
                p3 = prod_pool.tile([PT, 400], SEG_DT, tag="p3")
                nc.gpsimd.tensor_tensor(p3[:], p2s[0][:], p2s[1][:],
                                        op=OP.mult)
                with nc.allow_low_precision(reason="u16 bit sum in f32"):
                    nc.gpsimd.tensor_reduce(
                        combo[:, i:i + 1], p3[:].bitcast(U16),
                        axis=XA.XYZWC, op=OP.add)

            nc.sync.dma_start(outv[:], combo[:])

    nc.compile()
    return nc


def _get_program(k_pad: int, n_chunks: int):
    key = (k_pad, n_chunks)
    if key not in _PROG_CACHE:
        _PROG_CACHE[key] = _build_program(k_pad, n_chunks)
    return _PROG_CACHE[key]


# -------------------------------------------------------------------- kernel

def kernel(seg_pred: np.ndarray, bboxes: np.ndarray, batch_idx: np.ndarray,
           is_seg: np.ndarray) -> np.ndarray:
    seg_pred = np.asarray(seg_pred, dtype=np.float32)
    bboxes = np.asarray(bboxes, dtype=np.float32)
    batch_idx = np.asarray(batch_idx)
    is_seg = np.asarray(is_seg).astype(bool)
    assert seg_pred.shape == (B, 1, H, W), seg_pred.shape

    x1, y1, x2, y2 = _box_coords(bboxes, H, W)
    per_img = [[] for _ in range(B)]
    for m in range(bboxes.shape[0]):
        bi = int(batch_idx[m])
        if 0 <= bi < B:
            per_img[bi].append((int(x1[m]), int(y1[m]), int(x2[m]), int(y2[m])))

    has_box = np.array([len(p) > 0 for p in per_img])
    eff = (~is_seg) & has_box
    if not (eff.any() and not is_seg.all()):
        return np.float32(0.0)

    rects = [_disjoint_rects(p) for p in per_img]
    k_max = max(len(r) for r in rects)
    n_chunks = max(1, math.ceil(k_max / PT))
    # multiples of 32: engine partition-offset reads of the colhit tile
    # must start at a 32-partition boundary
    k_pad = min(PT, max(32, math.ceil(k_max / n_chunks / 32) * 32))
    V = IPC * n_chunks

    # per-core input arrays
    in_maps = []
    for core in range(N_CORES):
        imgs = [core * IPC + i for i in range(IPC)]
        # [i, p, t*W+w] layout: each SBUF partition's payload is contiguous
        seg_arr = np.ascontiguousarray(
            seg_pred[imgs, 0].reshape(IPC, NT, PT, W).transpose(0, 2, 1, 3)
            .reshape(IPC, PT, NW).astype(SEG_NP))
        rh_arr = np.zeros((PT, V * NT * k_pad), SEG_NP)
        ch_arr = np.zeros((V * k_pad, W), SEG_NP)
        for i, b in enumerate(imgs):
            for r, (rx1, rx2, ry1, ry2) in enumerate(rects[b]):
                c, rr = divmod(r, k_pad)
                v = i * n_chunks + c
                ch_arr[v * k_pad + rr, rx1:rx2 + 1] = 1
                for t in range(NT):
                    lo, hi = max(ry1, t * PT), min(ry2, t * PT + PT - 1)
                    if lo <= hi:
                        col = (v * NT + t) * k_pad + rr
                        rh_arr[lo - t * PT:hi - t * PT + 1, col] = 1
        in_maps.append({"seg": seg_arr, "rowhit": rh_arr, "colhit": ch_arr})

    nc = _get_program(k_pad, n_chunks)
    global LAST_RESULT
    res = run_bass_kernel_spmd(nc, in_maps, list(range(N_CORES)), trace=TRACE)
    LAST_RESULT = res

    # host reduction in float64
    total = 0.0
    log2_full = math.log(2.0) * H * W
    n_comp = PT * (NW // 8)     # compressed elements per image
    for core in range(N_CORES):
        ov = res.results[core]["outv"][0].astype(np.float64)  # [IPC + V]
        for i in range(IPC):
            b = core * IPC + i
            if eff[b]:
                sp_sum = -(ov[i] - n_comp * _KC) * _LN2_128
                m_sum = ov[IPC + i * n_chunks:IPC + (i + 1) * n_chunks].sum()
                total += sp_sum - m_sum
            else:
                total += log2_full
    loss = WEIGHT * total / (B * H * W)
    return np.float32(loss)
